# revision 2
# baseline (speedup 1.0000x reference)
"""MoE location-expert router kernel for Trainium2 (8 NeuronCores), v3.

Same math/host routing as v2 (vocab-sharded, tokens sorted by expert,
padded to 128). Device-side changes vs v2, from trace evidence:

  - v2 regression root cause: per-expert 8x1MB weight-prefetch bursts
    saturate the sync HWDGE ring; out DMAs block on ring slots, casts
    back up, PE stalls on PSUM WAR (5us gaps), and the final tiles'
    out DMAs serialize into a 45us tail.
  - v3: ONE [128, 4000] fp16 staging tile per token tile, 8 casts into
    it, a single contiguous 1MB out DMA issued on the SCALAR engine's
    separate HWDGE ring (37 out DMAs total vs 296).
  - Weight prefetch for expert e+1 is spread one kc-slice per token
    tile of expert e (no burst). Expert 0's head load alternates
    sync/scalar rings for 2x fill rate before any out DMAs exist.

v4 (trace-driven fixes on v3):
  - xe DMAs issued ONE EXPERT AHEAD (xpool bufs=3), before any W
    prefetch slices, so the x-token chain never lands near a boundary
    and Tile's coarse DMA-lane waits on the vector queue (head-of-line
    blocking of casts -> 17us PE stall) are satisfied early.
  - out DMA split in vocab halves across the sync and scalar HWDGE
    rings: halves the per-ring write load and the kernel tail.
  - Expert processing order puts a tiny-remainder expert last so the
    final out DMA is a few rows, shrinking the tail drain.
"""

import os

import numpy as np

import concourse.bacc as bacc
import concourse.bass as bass
import concourse.mybir as mybir
import concourse.tile as tile
from concourse.bass_utils import run_bass_kernel_spmd

E = 8          # experts
D = 1024       # d_model
V = 32000      # vocab
B = 4096       # tokens
NCORES = 8
VS = V // NCORES       # vocab slice per core (4000)
KT = 128               # contraction tile (partition dim)
KC = D // KT           # 8 K-chunks
MT = 128               # token tile (PSUM partition dim)
NT = 500               # vocab tile (moving free dim, <=512 for one PSUM bank)
NV = VS // NT          # 8 vocab tiles per core

MODE = os.environ.get("KERNEL_MODE", "fp16")

_program_cache = {}


def _build_program(pad_counts, counts, mode):
    """Trace the SPMD Tile program for the given per-expert padded counts."""
    if mode == "fp16":
        io_dt = mybir.dt.float16
    elif mode == "bf16":
        io_dt = mybir.dt.bfloat16
    else:
        io_dt = mybir.dt.float32r

    m_total = int(sum(pad_counts))
    nc = bacc.Bacc("TRN2", target_bir_lowering=False, debug=False,
                   enable_asserts=False, num_devices=NCORES)

    xT = nc.dram_tensor("xT", [D, m_total], io_dt, kind="ExternalInput").ap()
    wT = nc.dram_tensor("wT", [E, D, VS], io_dt, kind="ExternalInput").ap()
    out_dt = mybir.dt.float16 if mode == "fp16" else mybir.dt.float32
    out = nc.dram_tensor("out", [B, VS], out_dt, kind="ExternalOutput").ap()

    xT_r = xT.rearrange("(kc p) m -> p kc m", p=KT)
    live_experts = [e for e in range(E) if int(pad_counts[e]) > 0]
    # process experts in descending-remainder order: the final tile's
    # out DMA (and so the kernel tail) covers only the smallest
    # remainder's rows
    live_experts.sort(key=lambda e: -(int(counts[e]) % MT or MT))

    with tile.TileContext(nc) as tc:
        with (
            tc.tile_pool(name="xp", bufs=2) as xpool,
            tc.tile_pool(name="wp", bufs=2) as wpool,
            tc.tile_pool(name="op", bufs=8) as opool,
            tc.tile_pool(name="ps", bufs=8, space="PSUM") as pspool,
        ):
            we_tiles = {}

            def ensure_we(ei):
                # ei indexes live_experts
                if ei >= len(live_experts):
                    return None, None
                if ei not in we_tiles:
                    e = live_experts[ei]
                    wt = wpool.tile([KT, KC, VS], io_dt, tag="w",
                                    name=f"we{e}")
                    we_tiles[ei] = (wt, wT[e].rearrange("(kc p) v -> p kc v",
                                                        p=KT))
                return we_tiles[ei]

            pad_offs = {}
            val_offs = {}
            po = vo = 0
            for e in range(E):
                pad_offs[e], val_offs[e] = po, vo
                po += int(pad_counts[e])
                vo += int(counts[e])

            xe_tiles = {}

            def ensure_xe(ei):
                if ei >= len(live_experts):
                    return None
                if ei not in xe_tiles:
                    e = live_experts[ei]
                    pe = int(pad_counts[e])
                    xt = xpool.tile([KT, KC, pe], io_dt, tag="x",
                                    name=f"xe{e}")
                    # expert 0's tokens ride the scalar ring so W0[kc0]
                    # (sync) and xe0 transfer concurrently at the head
                    eng = nc.scalar if ei == 0 else nc.sync
                    eng.dma_start(
                        out=xt[:, :, :],
                        in_=xT_r[:, :, pad_offs[e]:pad_offs[e] + pe],
                    )
                    xe_tiles[ei] = xt
                return xe_tiles[ei]

            # first expert's tokens BEFORE its weights: the first matmul
            # needs xe0 + W0[kc0]; everything else pipelines behind
            ensure_xe(0)
            we0, wT0 = ensure_we(0)
            for kc in range(KC):
                # kc0 leads on sync (first matmul needs it); xe0 is on
                # scalar; remaining slices alternate
                eng = nc.sync if kc % 2 == 0 else nc.scalar
                eng.dma_start(out=we0[:, kc, :], in_=wT0[:, kc, :])

            for ei, e in enumerate(live_experts):
                pe = int(pad_counts[e])
                pad_off, val_off = pad_offs[e], val_offs[e]
                xe = ensure_xe(ei)
                ensure_xe(ei + 1)   # next expert's tokens, ahead of its W
                we, _ = ensure_we(ei)
                nxt_we, nxt_wT = ensure_we(ei + 1)
                ntiles = pe // MT
                per = -(-KC // ntiles)  # kc slices to prefetch per tile
                kc_next = 0
                for t in range(ntiles):
                    # spread next expert's weight prefetch across tiles
                    if nxt_we is not None:
                        for _ in range(per):
                            if kc_next < KC:
                                nc.sync.dma_start(
                                    out=nxt_we[:, kc_next, :],
                                    in_=nxt_wT[:, kc_next, :],
                                )
                                kc_next += 1
                    psts = [pspool.tile([MT, NT], mybir.dt.float32, tag="ps",
                                        name=f"ps{v}")
                            for v in range(NV)]
                    for kc in range(KC):
                        lhsT = xe[:, kc, t * MT:(t + 1) * MT]
                        for v in range(NV):
                            nc.tensor.matmul(
                                psts[v][:, :], lhsT,
                                we[:, kc, v * NT:(v + 1) * NT],
                                start=(kc == 0), stop=(kc == KC - 1),
                            )
                    valid = min(MT, int(counts[e]) - t * MT)
                    r0 = val_off + t * MT
                    # half-vocab staging tiles (4KB/partition, bufs=8):
                    # doubles the cast->out-DMA WAR slack vs whole-tile
                    # staging and halves each HBM write burst. Out DMAs
                    # stay off the sync ring (loads only).
                    for h in range(2):
                        oth = opool.tile([MT, VS // 2], out_dt, tag="o",
                                         name=f"ot{h}")
                        for v in range(NV // 2):
                            vv = h * (NV // 2) + v
                            nc.vector.tensor_copy(
                                oth[:, v * NT:(v + 1) * NT], psts[vv][:, :])
                        nc.scalar.dma_start(
                            out=out[r0:r0 + valid,
                                    h * (VS // 2):(h + 1) * (VS // 2)],
                            in_=oth[:valid, :],
                        )
    nc.compile()
    return nc, m_total


def _get_program(counts, mode):
    pad_counts = tuple(int(-(-c // MT) * MT) for c in counts)
    key = (pad_counts, tuple(int(c) for c in counts), mode)
    if key not in _program_cache:
        _program_cache[key] = _build_program(pad_counts, counts, mode)
    return pad_counts, _program_cache[key]


def _prepare(x, pointer_addresses, W, mode):
    idx = (np.asarray(pointer_addresses).astype(np.int64) % E).astype(np.int32)
    counts = np.bincount(idx, minlength=E)
    order = np.argsort(idx, kind="stable")
    pad_counts, (nc, m_total) = _get_program(tuple(counts), mode)

    np_dt = np.dtype("float32")
    if mode == "fp16":
        np_dt = np.dtype(np.float16)
    elif mode == "bf16":
        import ml_dtypes
        np_dt = np.dtype(ml_dtypes.bfloat16)

    x = np.asarray(x, dtype=np.float32)
    xs = x[order]                      # [B, D] sorted by expert
    x_pad = np.zeros((m_total, D), dtype=np_dt)
    row = 0
    srow = 0
    for e in range(E):
        c = int(counts[e])
        x_pad[row:row + c] = xs[srow:srow + c]
        row += int(pad_counts[e])
        srow += c
    xT = np.ascontiguousarray(x_pad.T)  # [D, m_total]

    W = np.asarray(W)
    wts = []
    for c in range(NCORES):
        Wc = W[:, c * VS:(c + 1) * VS, :]                 # [E, VS, D] view
        WTc = np.ascontiguousarray(Wc.transpose(0, 2, 1))  # [E, D, VS]
        if mode in ("fp16", "bf16"):
            WTc = WTc.astype(np_dt)
        wts.append(WTc)
    return idx, order, nc, xT, wts


def _run(x, pointer_addresses, W, b, trace=False, mode=None):
    mode = mode or MODE
    idx, order, nc, xT, wts = _prepare(x, pointer_addresses, W, mode)
    in_maps = [{"xT": xT, "wT": wts[c]} for c in range(NCORES)]
    kw = {}
    if trace:
        kw = dict(trace=True, trace_cores=[0])
    res = run_bass_kernel_spmd(nc, in_maps, list(range(NCORES)), **kw)

    out = np.empty((B, V), dtype=np.float32)
    for c in range(NCORES):
        out[order, c * VS:(c + 1) * VS] = res.results[c]["out"]

    b = np.asarray(b)
    if b.any():
        for e in range(E):
            out[idx == e] += b[e].astype(np.float32)
    return out, res


def kernel(x, pointer_addresses, W, b):
    out, _ = _run(x, pointer_addresses, W, b, trace=False)
    return out
